# revision 34
# baseline (speedup 1.0000x reference)
"""Trainium2 Bass kernel for DocumentBertScoringLoss (B=8192).

loss = MSE(p, g) + MR(p, g) + SIM(p, g), returned as shape-(1,) fp32.

Key identity (verified numerically): summing the margin-ranking hinge over
all ordered pairs (m, n), with r = sign(dp) (or -sign(dg) at ties, which
does not matter because r*dp = 0 there),

    sum max(0, 0.1 - r*dp) = 0.1*B^2 - 2 * sum clamp(p_m - p_n, 0, 0.1)

so the whole BxB hinge reduces to one clamp per pair.  Per device (row
stripe of 1024 rows), partition p / chunk c holds row value s1 = p_i and
the full prediction vector is broadcast along the free dim (X, fp16).
One chained DVE tensor_scalar computes h = min(max(X, s1), s1 + 0.1)
= s1 + clamp(p_n - p_m, 0, 0.1) (by (m,n) symmetry of the full double
sum the sign of the difference does not matter).  h tiles are summed by
the PE (ones-matmul accumulated in PSUM) and by the scalar engine
(Identity activation with accum_out); 8192*s1 is subtracted at the end.

Sharding: rows of the pairwise matrix, 1024 per core; predictions /
correct_output replicated.  Each core outputs its additive contribution
c_k; the host gather is a plain sum of the 8 scalars (the "all-reduce").
"""

import numpy as np

import concourse.bass as bass
import concourse.bacc as bacc
import concourse.mybir as mybir
from concourse.bass_utils import run_bass_kernel_spmd
from concourse.tile import TileContext
from concourse.alu_op_type import AluOpType

B = 8192
NCORES = 8
ROWS_PER_CORE = B // NCORES          # 1024
NCHUNK = ROWS_PER_CORE // 128        # 8 row chunks of 128 partitions
HALF = 4096                          # column tile width for the main pass
NHALF = B // HALF                    # 2
MR_BIAS = 0.1

# Column split inside each half-tile: [0, C_PE) reduced on the PE via
# ones-matmul, [C_PE, HALF) reduced on the scalar engine via accum_out.
C_PE = 2816
N_WARM = 8
MM_N = 512                           # PSUM bank limit for fp32 out

F32 = mybir.dt.float32
F16 = mybir.dt.float16

_CACHED = {}


def _tt(nc, out, in0, in1, op):
    # tensor_tensor via the TensorScalarPtr ISA struct ((0 + in0) op in1):
    # the gen3 TensorTensor struct only carries one sync-wait slot, which
    # the Tile scheduler can exceed; TSP carries more.
    nc.vector.scalar_tensor_tensor(out, in0, 0.0, in1, AluOpType.add, op)



def _build_nc():
    nc = bacc.Bacc("TRN2", target_bir_lowering=False, debug=False, num_devices=NCORES)

    pred_d = nc.dram_tensor("predictions", [B], F32, kind="ExternalInput")
    g_d = nc.dram_tensor("correct_output", [B], F32, kind="ExternalInput")
    prow_d = nc.dram_tensor("p_rows", [ROWS_PER_CORE], F32, kind="ExternalInput")
    out_d = nc.dram_tensor("out", [1], F32, kind="ExternalOutput")

    AF = mybir.ActivationFunctionType

    with TileContext(nc) as tc:
        with (
            tc.tile_pool(name="const", bufs=1) as cpool,
            tc.tile_pool(name="hbuf", bufs=3) as hpool,
            tc.tile_pool(name="psum", bufs=1, space="PSUM") as ppool,
        ):
            # ---- persistent tiles ----
            xbf = cpool.tile([128, B], F16, name="xbf")
            pred32 = cpool.tile([128, B // 128], F32, name="pred32")
            g32 = cpool.tile([128, B // 128], F32, name="g32")
            prow = cpool.tile([128, NCHUNK], F32, name="prow")
            s2 = cpool.tile([128, NCHUNK], F32, name="s2")
            ones_bf = cpool.tile([128, 1], F16, name="ones_bf")
            zero_bf = cpool.tile([128, 1], F16, name="zero_bf")
            ones_f32 = cpool.tile([128, 1], F32, name="ones_f32")
            stacked = cpool.tile([128, 6], F32, name="stacked")
            d_tile = cpool.tile([128, B // 128], F32, name="d_tile")
            junk_sq = cpool.tile([128, B // 128], F32, name="junk_sq")
            junk_stt = cpool.tile([128, B // 128], F32, name="junk_stt")
            sc = cpool.tile([1, 16], F32, name="sc")
            out_sb = cpool.tile([1, 1], F32, name="out_sb")

            psum_main = ppool.tile([128, MM_N], F32, name="psum_main")
            psum_warm = ppool.tile([128, MM_N], F32, name="psum_warm")
            psum_small = ppool.tile([128, 8], F32, name="psum_small")
            psum_acc = ppool.tile([128, NCHUNK * NHALF], F32, name="psum_acc")

            # ---- input DMAs ----
            pred_ap = pred_d[:]
            nc.sync.dma_start(prow, prow_d[:].rearrange("(p c) -> p c", p=128))
            nc.sync.dma_start(pred32, pred_ap.rearrange("(p c) -> p c", p=128))
            nc.sync.dma_start(g32, g_d[:].rearrange("(p c) -> p c", p=128))

            # Broadcast predictions along partitions into X [128, B] fp16.
            # Column blocks pipeline the transfer so the first clamp starts
            # early.  Blocks 0-1 cast-broadcast straight from the f32 input
            # (lowest latency); the rest broadcast from a 16KB fp16 DRAM
            # scratch written on-chip, which halves the HBM read volume of
            # the 128x re-read (all 8 cores broadcast simultaneously, so
    
            # HBM pressure is the multi-core risk).
            XBLK = 1024
            NDIRECT = 4
            pred16 = cpool.tile([128, B // 128], F16, name="pred16")
            nc.vector.tensor_copy(pred16, pred32)
            scratch16 = nc.dram_tensor("pred16_scratch", [B], F16, kind="Internal")
            nc.gpsimd.dma_start(
                scratch16[:].rearrange("(p c) -> p c", p=128), pred16
            )
            for j in range(B // XBLK):
                if j < NDIRECT:
                    nc.gpsimd.dma_start(
                        xbf[:, j * XBLK:(j + 1) * XBLK],
                        pred_ap[j * XBLK:(j + 1) * XBLK].partition_broadcast(128),
                    )
                else:
                    nc.gpsimd.dma_start(
                        xbf[:, j * XBLK:(j + 1) * XBLK],
                        scratch16[j * XBLK:(j + 1) * XBLK].partition_broadcast(128),
                    )

            zeros1 = cpool.tile([128, 1], F32, name="zeros1")
            nc.vector.memset(zeros1, 0.0)
            # Dummy sqrt issued first so the single act-table load picks a
            # set containing sqrt+identity+square (avoids a second ~1.3us
            # LoadActFuncSet right before the final scalar chain).
            warm_sqrt = cpool.tile([1, 1], F32, name="warm_sqrt")
            nc.scalar.activation(warm_sqrt, zeros1[0:1, :], AF.Sqrt,
                                 bias=zeros1[0:1, :])
            nc.vector.memset(ones_bf, 1.0)
            nc.vector.memset(zero_bf, 0.0)
            nc.vector.memset(ones_f32, 1.0)
            nc.vector.tensor_scalar(s2, prow, MR_BIAS, None, AluOpType.add)

            # PE warm-up: dense dummy matmuls from t~0.5us keep the HAM
            # activity window busy so the real matmul stream runs at the
            # warm clock from its first instruction.
            junk_bf = cpool.tile([128, MM_N], F16, name="junk_bf")
            nc.vector.memset(junk_bf, 0.0)
            for _w in range(N_WARM):
                nc.tensor.matmul(
                    psum_warm[0:1, 0:MM_N], ones_bf, junk_bf,
                    start=True, stop=True,
                )

            # ---- main pass: h = min(max(X, s1), s1 + 0.1) ----
            C_ACT = HALF - C_PE
            mm_idx = 0
            acc_slots = []
            # half-outer loop: all chunks of column-half 0 run while the
            # second half of the broadcast is still in flight.
            for hh in range(NHALF):
                x_half = xbf[:, hh * HALF:(hh + 1) * HALF]
                for cp in range(NCHUNK // 2):
                    last_pair = (hh == NHALF - 1 and cp == NCHUNK // 2 - 1)
                    # ACT tile shared by two consecutive chunks: halves the
                    # per-instruction overhead of the ACT accumulation.
                    h_act = None
                    if not last_pair:
                        h_act = hpool.tile(
                            [128, 2 * C_ACT], F16, tag="h_act", name="h_act",
                            bufs=4,
                        )
                    c_pe_here = HALF if last_pair else C_PE
                    for ci in range(2):
                        c = 2 * cp + ci
                        # The first chunks use narrower clamp pieces so the
                        # PE starts as soon as the first broadcast blocks
                        # land, instead of waiting for the full half.
                        if hh == 0 and cp == 0 and ci == 0:
                            cuts = [0, 1024, 2048, C_PE]
                        elif hh == 0 and cp == 0:
                            cuts = [0, 2048, C_PE]
                        else:
                            cuts = [0, c_pe_here]
                        for p0, p1 in zip(cuts, cuts[1:]):
                            h_pe = hpool.tile(
                                [128, c_pe_here], F16, tag="h_pe", name="h_pe",
                                bufs=5,
                            )
                            nc.vector.tensor_scalar(
                                h_pe[:, 0:p1 - p0],
                                x_half[:, p0:p1],
                                prow[:, c:c + 1],
                                s2[:, c:c + 1],
                                AluOpType.max,
                                AluOpType.min,
                            )
                            for n0 in range(p0, p1, MM_N):
                                n1 = min(n0 + MM_N, p1)
                                nc.tensor.matmul(
                                    psum_main[0:1, 0:n1 - n0],
                                    ones_bf,
                                    h_pe[:, n0 - p0:n1 - p0],
                                    start=(mm_idx == 0),
                                    stop=False,
                                    skip_group_check=True,
                                )
                                mm_idx += 1
                        if not last_pair:
                            nc.vector.tensor_scalar(
                                h_act[:, ci * C_ACT:(ci + 1) * C_ACT],
                                x_half[:, C_PE:HALF],
                                prow[:, c:c + 1],
                                s2[:, c:c + 1],
                                AluOpType.max,
                                AluOpType.min,
                            )
                    if not last_pair:
                        a_slot = hpool.tile([128, 1], F32, tag="a_slot",
                                            bufs=NCHUNK, name="a_slot")
                        acc_slots.append(a_slot)
                        nc.scalar.activation(
                            h_act,
                            h_act,
                            AF.Identity,
                            bias=zeros1,
                            accum_out=a_slot,
                        )

            # close the psum_main accumulation group
            nc.tensor.matmul(
                psum_main[0:1, 0:1],
                ones_bf,
                zero_bf,
                start=False,
                stop=True,
                skip_group_check=True,
            )

            # ---- small terms ----
            # Sum the per-iteration ACT accumulators on the PE: one matmul
            # per slot accumulating into one PSUM scalar.
            for si_, a_slot in enumerate(acc_slots):
                nc.tensor.matmul(
                    psum_acc[0:1, 0:1],
                    ones_f32,
                    a_slot,
                    start=(si_ == 0),
                    stop=(si_ == len(acc_slots) - 1),
                )
            nc.vector.tensor_reduce(
                stacked[:, 1:2], prow, mybir.AxisListType.X, AluOpType.add
            )
            # Copies give each DMA'd tile a single-wait first consumer on
            # DVE; downstream DVE ops then read same-engine tiles only
            # (the TT/STT ISA structs carry a single sync-wait slot).
            pred_c = cpool.tile([128, B // 128], F32, name="pred_c")
            g_c = cpool.tile([128, B // 128], F32, name="g_c")
            nc.vector.tensor_copy(pred_c, pred32)
            nc.vector.tensor_copy(g_c, g32)
            _tt(nc, d_tile, pred_c, g_c, AluOpType.subtract)
            sq_acc = cpool.tile([128, 1], F32, name="sq_acc")
            nc.scalar.activation(
                junk_sq, d_tile, AF.Square, bias=zeros1, accum_out=sq_acc
            )
            nc.vector.tensor_copy(stacked[:, 2:3], sq_acc)
            nc.vector.scalar_tensor_tensor(
                junk_stt, pred_c, 1.0, g_c, AluOpType.mult, AluOpType.mult,
                accum_out=stacked[:, 3:4],
            )
            nc.vector.scalar_tensor_tensor(
                junk_stt, pred_c, 1.0, pred_c, AluOpType.mult, AluOpType.mult,
                accum_out=stacked[:, 4:5],
            )
            nc.vector.scalar_tensor_tensor(
                junk_stt, g_c, 1.0, g_c, AluOpType.mult, AluOpType.mult,
                accum_out=stacked[:, 5:6],
            )

            # partition reduction: [1, 6] = ones^T @ stacked
            nc.tensor.matmul(
                psum_small[0:1, 0:6], ones_f32, stacked, start=True, stop=True
            )

            # ---- final scalar assembly (partition 0) ----
            smalls = cpool.tile([1, 6], F32, name="smalls")
            nc.vector.tensor_copy(smalls, psum_small[0:1, 0:6])
            t_act = sc[0:1, 13:14]
            nc.vector.tensor_copy(t_act, psum_acc[0:1, 0:1])
            p_sum = smalls[0:1, 1:2]
            sq = smalls[0:1, 2:3]
            dot = smalls[0:1, 3:4]
            pp = smalls[0:1, 4:5]
            gg = smalls[0:1, 5:6]

            tpe = sc[0:1, 0:1]
            nc.vector.tensor_reduce(
                tpe, psum_main[0:1, 0:MM_N], mybir.AxisListType.X, AluOpType.add
            )
            s_all = sc[0:1, 1:2]
            _tt(nc, s_all, tpe, t_act, AluOpType.add)
            corr = sc[0:1, 2:3]
            nc.vector.tensor_scalar(corr, p_sum, float(B), None, AluOpType.mult)
            s_clamp = sc[0:1, 3:4]
            _tt(nc, s_clamp, s_all, corr, AluOpType.subtract)
            # mr_part = s_clamp * (-2/B^2) + 0.1/8
            mr_part = sc[0:1, 4:5]
            nc.vector.tensor_scalar(
                mr_part, s_clamp, -2.0 / (float(B) * float(B)),
                MR_BIAS / NCORES, AluOpType.mult, AluOpType.add,
            )
            mse_part = sc[0:1, 5:6]
            nc.vector.tensor_scalar(
                mse_part, sq, 1.0 / (float(B) * NCORES), None, AluOpType.mult
            )
            prod = sc[0:1, 6:7]
            _tt(nc, prod, pp, gg, AluOpType.mult)
            denom = sc[0:1, 7:8]
            nc.scalar.activation(denom, prod, AF.Sqrt, bias=zeros1[0:1, :])
            dmax = sc[0:1, 8:9]
            nc.vector.tensor_scalar(dmax, denom, 1e-8, None, AluOpType.max)
            inv = sc[0:1, 9:10]
            nc.vector.reciprocal(inv, dmax)
            sims = sc[0:1, 10:11]
            _tt(nc, sims, dot, inv, AluOpType.mult)
            # sim_part = (1 - sims)/8
            sim_part = sc[0:1, 11:12]
            nc.vector.tensor_scalar(
                sim_part, sims, -1.0 / NCORES, 1.0 / NCORES,
                AluOpType.mult, AluOpType.add,
            )
            acc1 = sc[0:1, 12:13]
            _tt(nc, acc1, mr_part, mse_part, AluOpType.add)
            _tt(nc, out_sb, acc1, sim_part, AluOpType.add)

            nc.sync.dma_start(out_d[None, :], out_sb)

    nc.compile()
    return nc


def kernel(predictions: np.ndarray, correct_output: np.ndarray) -> np.ndarray:
    pred = np.ascontiguousarray(np.asarray(predictions, dtype=np.float32))
    g = np.ascontiguousarray(np.asarray(correct_output, dtype=np.float32))

    if "nc" not in _CACHED:
        _CACHED["nc"] = _build_nc()
    nc = _CACHED["nc"]

    in_maps = []
    for k in range(NCORES):
        in_maps.append(
            {
                "predictions": pred,
                "correct_output": g,
                "p_rows": np.ascontiguousarray(
                    pred[k * ROWS_PER_CORE:(k + 1) * ROWS_PER_CORE]
                ),
            }
        )

    res = run_bass_kernel_spmd(nc, in_maps, core_ids=list(range(NCORES)))
    total = np.float32(0.0)
    for r in res.results:
        total = np.float32(total + np.float32(r["out"][0]))
    return np.array([total], dtype=np.float32)


if __name__ == "__main__":
    rng = np.random.default_rng(0)
    p = rng.standard_normal(B).astype(np.float32)
    g = rng.standard_normal(B).astype(np.float32)
    print(kernel(p, g))



# revision 61
# speedup vs baseline: 1.0414x; 1.0414x over previous
"""Trainium2 Bass kernel for DocumentBertScoringLoss (B=8192).

loss = MSE(p, g) + MR(p, g) + SIM(p, g), returned as shape-(1,) fp32.

Key identity (verified numerically): summing the margin-ranking hinge over
all ordered pairs (m, n), with r = sign(dp) (or -sign(dg) at ties, which
does not matter because r*dp = 0 there),

    sum max(0, 0.1 - r*dp) = 0.1*B^2 - 2 * sum clamp(p_m - p_n, 0, 0.1)

so the whole BxB hinge reduces to one clamp per pair.  Per device (row
stripe of 1024 rows), partition p / chunk c holds row value s1 = p_i and
the full prediction vector is broadcast along the free dim (X, fp16).
One chained DVE tensor_scalar computes h = min(max(X, s1), s1 + 0.1)
= s1 + clamp(p_n - p_m, 0, 0.1) (by (m,n) symmetry of the full double
sum the sign of the difference does not matter).  h tiles are summed by
the PE (ones-matmul accumulated in PSUM) and by the scalar engine
(Identity activation with accum_out); 8192*s1 is subtracted at the end.

Sharding: rows of the pairwise matrix, 1024 per core; predictions /
correct_output replicated.  Each core outputs its additive contribution
c_k; the host gather is a plain sum of the 8 scalars (the "all-reduce").
"""

import numpy as np

import concourse.bass as bass
import concourse.bacc as bacc
import concourse.mybir as mybir
from concourse.bass_utils import run_bass_kernel_spmd
from concourse.tile import TileContext
from concourse.alu_op_type import AluOpType

B = 8192
NCORES = 8
ROWS_PER_CORE = B // NCORES          # 1024
NCHUNK = ROWS_PER_CORE // 128        # 8 row chunks of 128 partitions
HALF = 4096                          # column tile width for the main pass
NHALF = B // HALF                    # 2
MR_BIAS = 0.1

# Column split inside each half-tile: [0, C_PE) reduced on the PE via
# ones-matmul, [C_PE, HALF) reduced on the scalar engine via accum_out.
C_PE = 2816
N_WARM = 8
MM_N = 512                           # PSUM bank limit for fp32 out

F32 = mybir.dt.float32
F16 = mybir.dt.float16

_CACHED = {}


def _tt(nc, out, in0, in1, op):
    # tensor_tensor via the TensorScalarPtr ISA struct ((0 + in0) op in1):
    # the gen3 TensorTensor struct only carries one sync-wait slot, which
    # the Tile scheduler can exceed; TSP carries more.
    nc.vector.scalar_tensor_tensor(out, in0, 0.0, in1, AluOpType.add, op)



def _build_nc():
    nc = bacc.Bacc("TRN2", target_bir_lowering=False, debug=False, num_devices=NCORES)

    pred_d = nc.dram_tensor("predictions", [B], F32, kind="ExternalInput")
    g_d = nc.dram_tensor("correct_output", [B], F32, kind="ExternalInput")
    # p_rows arrives transposed [8, 128] (prow_t[c, p] = row value of
    # chunk c / partition p): an [8, 128] DMA is 8 fat descriptors
    # (~0.8us) instead of the 128 tiny ones a direct [128, 8] fill needs
    # (~3.3us, which gated the first clamp).  A tiny PE matmul against an
    # 8x8 identity transposes it on chip.
    prow_d = nc.dram_tensor("p_rows", [NCHUNK, 128], F32, kind="ExternalInput")
    eye_d = nc.dram_tensor("eye8", [NCHUNK, NCHUNK], F32, kind="ExternalInput")
    out_d = nc.dram_tensor("out", [2], F32, kind="ExternalOutput")

    AF = mybir.ActivationFunctionType

    with TileContext(nc) as tc:
        with (
            tc.tile_pool(name="const", bufs=1) as cpool,
            tc.tile_pool(name="hbuf", bufs=3) as hpool,
            tc.tile_pool(name="psum", bufs=1, space="PSUM") as ppool,
        ):
            # ---- persistent tiles ----
            xbf = cpool.tile([128, B], F16, name="xbf")
            pred32 = cpool.tile([128, B // 128], F32, name="pred32")
            g32 = cpool.tile([128, B // 128], F32, name="g32")
            prow = cpool.tile([128, NCHUNK], F32, name="prow")
            s2 = cpool.tile([128, NCHUNK], F32, name="s2")
            ones_bf = cpool.tile([128, 1], F16, name="ones_bf")
            ones_f32 = cpool.tile([128, 1], F32, name="ones_f32")
            stacked = cpool.tile([128, 6], F32, name="stacked")
            d_tile = cpool.tile([128, B // 128], F32, name="d_tile")
            junk_sq = cpool.tile([128, B // 128], F32, name="junk_sq")
            junk_stt = cpool.tile([128, B // 128], F32, name="junk_stt")
            sc = cpool.tile([1, 16], F32, name="sc")
            out_sb2 = cpool.tile([1, 2], F32, name="out_sb2")

            psum_main = ppool.tile([128, MM_N], F32, name="psum_main")
            psum_warm = ppool.tile([128, MM_N], F32, name="psum_warm")
            psum_small = ppool.tile([128, 8], F32, name="psum_small")
            psum_acc = ppool.tile([128, NCHUNK * NHALF], F32, name="psum_acc")

            # ---- input DMAs ----
            pred_ap = pred_d[:]
            prow_t = cpool.tile([NCHUNK, 128], F32, name="prow_t")
            nc.sync.dma_start(prow_t, prow_d[:, :])
            eye8 = cpool.tile([NCHUNK, NCHUNK], F32, name="eye8")
            nc.sync.dma_start(eye8, eye_d[:, :])
            # pred32/g32 trigger from the scalar engine's HWDGE queue so the
            # tiny prow_t/eye8 transfers (which gate the first clamp) are
            # not queued behind them.
            nc.scalar.dma_start(pred32, pred_ap.rearrange("(p c) -> p c", p=128))
            nc.scalar.dma_start(g32, g_d[:].rearrange("(p c) -> p c", p=128))

            # Broadcast predictions along partitions into X [128, B] fp16.
            # Column blocks pipeline the transfer so the first clamp starts
            # early.  Blocks 0-1 cast-broadcast straight from the f32 input
            # (lowest latency); the rest broadcast from a 16KB fp16 DRAM
            # scratch written on-chip, which halves the HBM read volume of
            # the 128x re-read (all 8 cores broadcast simultaneously, so
    
            # HBM pressure is the multi-core risk).
            XBLK = 1024
            NDIRECT = 4
            scratch16 = nc.dram_tensor("pred16_scratch", [B], F16, kind="Internal")
            for j in range(NDIRECT):
                nc.gpsimd.dma_start(
                    xbf[:, j * XBLK:(j + 1) * XBLK],
                    pred_ap[j * XBLK:(j + 1) * XBLK].partition_broadcast(128),
                )
            # DRAM->DRAM cast (f32 -> fp16) with no SBUF roundtrip and no
            # upstream dependency, queued after the latency-critical direct
            # blocks so the Q7 descriptor queue never head-of-line blocks.
            nc.gpsimd.dma_start(scratch16[:], pred_ap)
            for j in range(NDIRECT, B // XBLK):
                nc.gpsimd.dma_start(
                    xbf[:, j * XBLK:(j + 1) * XBLK],
                    scratch16[j * XBLK:(j + 1) * XBLK].partition_broadcast(128),
                )

            zeros1 = cpool.tile([128, 1], F32, name="zeros1")
            nc.vector.memset(zeros1, 0.0)

            # on-chip transpose of prow_t [8,128] -> prow [128,8] via a
            # matmul against a host-provided 8x8 identity (engines cannot
            # write single non-32-aligned partitions to build it on chip).
            psum_pr = ppool.tile([128, NCHUNK], F32, name="psum_pr")
            nc.tensor.matmul(psum_pr, prow_t, eye8, start=True, stop=True)
            nc.vector.tensor_copy(prow, psum_pr)

            # Dummy sqrt issued first so the single act-table load picks a
            # set containing sqrt+identity+square (avoids a second ~1.3us
            # LoadActFuncSet right before the final scalar chain).
            warm_sqrt = cpool.tile([1, 1], F32, name="warm_sqrt")
            nc.scalar.activation(warm_sqrt, zeros1[0:1, :], AF.Sqrt,
                                 bias=zeros1[0:1, :])
            nc.vector.memset(ones_bf, 1.0)
            nc.vector.memset(ones_f32, 1.0)
            nc.vector.tensor_scalar(s2, prow, MR_BIAS, None, AluOpType.add)

            # PE warm-up: dense dummy matmuls from t~0.5us keep the HAM
            # activity window busy so the real matmul stream runs at the
            # warm clock from its first instruction.
            junk_bf = cpool.tile([128, MM_N], F16, name="junk_bf")
            nc.vector.memset(junk_bf, 0.0)
            for _w in range(N_WARM):
                nc.tensor.matmul(
                    psum_warm[0:1, 0:MM_N], ones_bf, junk_bf,
                    start=True, stop=True,
                )

            # ---- main pass: h = min(max(X, s1), s1 + 0.1) ----
            C_ACT = HALF - C_PE
            # main-MM count: 14 normal half-chunks x ceil(C_PE/512) + chunk 6
            # (full width, 8 MMs); chunk 7 reduces on the DVE instead.
            mm_total = 14 * ((C_PE + MM_N - 1) // MM_N) + HALF // MM_N
            mm_idx = 0
            acc_slots = []
            # half-outer loop: all chunks of column-half 0 run while the
            # second half of the broadcast is still in flight.
            for hh in range(NHALF):
                x_half = xbf[:, hh * HALF:(hh + 1) * HALF]
                for cp in range(NCHUNK // 2):
                    last_pair = (hh == NHALF - 1 and cp == NCHUNK // 2 - 1)
                    # ACT tile shared by two consecutive chunks: halves the
                    # per-instruction overhead of the ACT accumulation.
                    h_act = None
                    if not last_pair:
                        h_act = hpool.tile(
                            [128, 2 * C_ACT], F16, tag="h_act", name="h_act",
                            bufs=4,
                        )
                    c_pe_here = HALF if last_pair else C_PE
                    for ci in range(2):
                        c = 2 * cp + ci
                        # The first chunks use narrower clamp pieces so the
                        # PE starts as soon as the first broadcast blocks
                        # land, instead of waiting for the full half.
                        if hh == 0 and cp == 0 and ci == 0:
                            cuts = [0, 1024, 2048, C_PE]
                        elif hh == 0 and cp == 0:
                            cuts = [0, 2048, C_PE]
                        else:
                            cuts = [0, c_pe_here]
                        for p0, p1 in zip(cuts, cuts[1:]):
                            h_pe = hpool.tile(
                                [128, p1 - p0], F16, tag="h_pe", name="h_pe",
                                bufs=5,
                            )
                            nc.vector.tensor_scalar(
                                h_pe[:, 0:p1 - p0],
                                x_half[:, p0:p1],
                                prow[:, c:c + 1],
                                s2[:, c:c + 1],
                                AluOpType.max,
                                AluOpType.min,
                            )
                            if last_pair and ci == 1:
                                # very last chunk: reduce on the DVE itself
                                # (tensor_scalar add with accum) so the tail
                                # skips both the PE matmuls and the 658ns
                                # single-lane PSUM reduce
                                a_slot = hpool.tile(
                                    [128, 1], F32, tag="a_slot",
                                    bufs=NCHUNK, name="a_slot",
                                )
                                acc_slots.append(a_slot)
                                nc.vector.tensor_scalar(
                                    h_pe, h_pe, 0.0, None,
                                    AluOpType.add, AluOpType.add,
                                    accum_out=a_slot,
                                )
                                continue
                            for n0 in range(p0, p1, MM_N):
                                n1 = min(n0 + MM_N, p1)
                                nc.tensor.matmul(
                                    psum_main[0:1, 0:n1 - n0],
                                    ones_bf,
                                    h_pe[:, n0 - p0:n1 - p0],
                                    start=(mm_idx == 0),
                                    stop=(mm_idx == mm_total - 1),
                                    skip_group_check=True,
                                )
                                mm_idx += 1
                        if not last_pair:
                            nc.vector.tensor_scalar(
                                h_act[:, ci * C_ACT:(ci + 1) * C_ACT],
                                x_half[:, C_PE:HALF],
                                prow[:, c:c + 1],
                                s2[:, c:c + 1],
                                AluOpType.max,
                                AluOpType.min,
                            )
                    if not last_pair:
                        a_slot = hpool.tile([128, 1], F32, tag="a_slot",
                                            bufs=NCHUNK, name="a_slot")
                        acc_slots.append(a_slot)
                        nc.scalar.activation(
                            h_act,
                            h_act,
                            AF.Identity,
                            bias=zeros1,
                            accum_out=a_slot,
                        )

            # ---- small terms ----
            # Sum the per-iteration ACT accumulators on the PE: one matmul
            # per slot accumulating into one PSUM scalar.
            for si_, a_slot in enumerate(acc_slots):
                nc.tensor.matmul(
                    psum_acc[0:1, 0:1],
                    ones_f32,
                    a_slot,
                    start=(si_ == 0),
                    stop=(si_ == len(acc_slots) - 1),
                )
            nc.vector.tensor_reduce(
                stacked[:, 1:2], prow, mybir.AxisListType.X, AluOpType.add
            )
            _tt(nc, d_tile, pred32, g32, AluOpType.subtract)
            sq_acc = cpool.tile([128, 1], F32, name="sq_acc")
            nc.scalar.activation(
                junk_sq, d_tile, AF.Square, bias=zeros1, accum_out=sq_acc
            )
            nc.vector.tensor_copy(stacked[:, 2:3], sq_acc)
            nc.vector.scalar_tensor_tensor(
                junk_stt, pred32, 1.0, g32, AluOpType.mult, AluOpType.mult,
                accum_out=stacked[:, 3:4],
            )
            nc.vector.scalar_tensor_tensor(
                junk_stt, pred32, 1.0, pred32, AluOpType.mult, AluOpType.mult,
                accum_out=stacked[:, 4:5],
            )
            nc.vector.scalar_tensor_tensor(
                junk_stt, g32, 1.0, g32, AluOpType.mult, AluOpType.mult,
                accum_out=stacked[:, 5:6],
            )

            # partition reduction: [1, 6] = ones^T @ stacked
            nc.tensor.matmul(
                psum_small[0:1, 0:6], ones_f32, stacked, start=True, stop=True
            )

            # ---- final scalar assembly (partition 0) ----
            smalls = cpool.tile([1, 6], F32, name="smalls")
            nc.vector.tensor_copy(smalls, psum_small[0:1, 0:6])
            t_act = sc[0:1, 13:14]
            nc.vector.tensor_copy(t_act, psum_acc[0:1, 0:1])
            p_sum = smalls[0:1, 1:2]
            sq = smalls[0:1, 2:3]
            dot = smalls[0:1, 3:4]
            pp = smalls[0:1, 4:5]
            gg = smalls[0:1, 5:6]

            tpe = sc[0:1, 0:1]
            nc.vector.tensor_reduce(
                tpe, psum_main[0:1, 0:MM_N], mybir.AxisListType.X, AluOpType.add
            )
            s_all = sc[0:1, 1:2]
            _tt(nc, s_all, tpe, t_act, AluOpType.add)
            corr = sc[0:1, 2:3]
            nc.vector.tensor_scalar(corr, p_sum, float(B), None, AluOpType.mult)
            s_clamp = sc[0:1, 3:4]
            _tt(nc, s_clamp, s_all, corr, AluOpType.subtract)

            mse_part = sc[0:1, 5:6]
            nc.vector.tensor_scalar(
                mse_part, sq, 1.0 / (float(B) * NCORES), None, AluOpType.mult
            )
            prod = sc[0:1, 6:7]
            _tt(nc, prod, pp, gg, AluOpType.mult)
            denom = sc[0:1, 7:8]
            nc.scalar.activation(denom, prod, AF.Sqrt, bias=zeros1[0:1, :])
            dmax = sc[0:1, 8:9]
            nc.vector.tensor_scalar(dmax, denom, 1e-8, None, AluOpType.max)
            inv = sc[0:1, 9:10]
            nc.vector.reciprocal(inv, dmax)
            sims = sc[0:1, 10:11]
            _tt(nc, sims, dot, inv, AluOpType.mult)
            # sim_part = (1 - sims)/8
            sim_part = sc[0:1, 11:12]
            nc.vector.tensor_scalar(
                sim_part, sims, -1.0 / NCORES, 1.0 / NCORES,
                AluOpType.mult, AluOpType.add,
            )
            # out[0] = mse/8 + (1-sim)/8 + 0.1/8 completes early (only
            # psum_small-dependent); out[1] = -2*S/B^2 is the tail-critical
            # value.  The host sum over 16 numbers is unchanged math.
            early = sc[0:1, 12:13]
            _tt(nc, early, mse_part, sim_part, AluOpType.add)
            nc.vector.tensor_scalar(
                out_sb2[0:1, 0:1], early, MR_BIAS / NCORES, None, AluOpType.add
            )
            nc.vector.tensor_scalar(
                out_sb2[0:1, 1:2], s_clamp, -2.0 / (float(B) * float(B)),
                None, AluOpType.mult,
            )
            nc.sync.dma_start(out_d[None, :], out_sb2)

    nc.compile()
    return nc


def kernel(predictions: np.ndarray, correct_output: np.ndarray) -> np.ndarray:
    pred = np.ascontiguousarray(np.asarray(predictions, dtype=np.float32))
    g = np.ascontiguousarray(np.asarray(correct_output, dtype=np.float32))

    if "nc" not in _CACHED:
        _CACHED["nc"] = _build_nc()
    nc = _CACHED["nc"]

    in_maps = []
    for k in range(NCORES):
        in_maps.append(
            {
                "predictions": pred,
                "correct_output": g,
                "p_rows": np.ascontiguousarray(
                    pred[k * ROWS_PER_CORE:(k + 1) * ROWS_PER_CORE]
                    .reshape(128, NCHUNK).T
                ),
                "eye8": np.eye(NCHUNK, dtype=np.float32),
            }
        )

    res = None
    last_exc = None
    for _attempt in range(3):
        try:
            res = run_bass_kernel_spmd(nc, in_maps, core_ids=list(range(NCORES)))
            break
        except Exception as e:  # transient NRT/axon device errors
            last_exc = e
            import time as _time
            _time.sleep(1.0)
    if res is None:
        raise last_exc
    total = np.float32(0.0)
    for r in res.results:
        total = np.float32(total + np.float32(r["out"][0]) + np.float32(r["out"][1]))
    return np.array([total], dtype=np.float32)


if __name__ == "__main__":
    rng = np.random.default_rng(0)
    p = rng.standard_normal(B).astype(np.float32)
    g = rng.standard_normal(B).astype(np.float32)
    print(kernel(p, g))

